# revision 2
# baseline (speedup 1.0000x reference)
"""MinLSTM fused kernel for Trainium2 (8 NeuronCores, SPMD).

Math: the reference applies cumlogsumexp over the sequence but only the LAST
timestep feeds the output head, so the scan collapses to a single logsumexp
reduction over sequence:

    log_h_last = log_f[S-1] + log(0.5 + sum_s exp(diff_s + log_g(h_s)))
    out = exp(log_h_last) @ w_out.T + b_out

with diff = softplus(-f) - softplus(-i) and per-token term

    exp(diff + log_g(h)) = (1 + e^{-f}) * sigmoid(i) * g(h)
                         = 1/4 * (1+e^{-f}) * (1+tanh(i/2)) * max(1+2h, 1+tanh(h/2))

which needs only {exp, tanh, copy} — all in the ACT `exp_and_others` table
(single table load). The device fuses the z = x @ w_in.T matmul (fp8
DoubleRow, fp32 PSUM accumulation) with the per-token nonlinearity and the
per-(batch, channel) partial sums. The host applies the exact last-token
correction in fp64 and runs the tiny [4,1024]x[1024,1024] output head.

Sharding: hidden-channel parallel — core c owns h-channels [c*128, (c+1)*128)
(i.e. 3 x 128 rows of w_in) and streams ALL 32768 tokens. This keeps the
per-core weight load to a single 384 KB DMA (vs streaming 3 MB of stripes),
makes every token block a full 512 (the moving-dim size where the PE is
stream-bound rather than LDWEIGHTS-bound), and leaves a single short
ACT->DVE drain after the last matmul. Per-core HBM traffic is 32 MB of fp8
x at ~195 GB/s sustained — well under the ~358 GB/s per-core ceiling.

Startup: weights and the first x block are DMA'd in kb-pair pieces on the
two HWDGE queues (w on scalar, x on sync) so the first real matmul can start
as soon as the first ~224 KB land, chasing the remaining pieces. A short
dummy-matmul burst opens the PE HAM clock gate early; the first real matmuls
still run partially cold (1.2 GHz) — cheaper than idling until warm.
"""

from contextlib import ExitStack

import ml_dtypes
import numpy as np

B, S, D, H = 4, 8192, 1024, 1024
N_CORES = 8
CH = H // N_CORES       # 128 h-channels per core
TOKS = B * S            # 32768 tokens, all streamed by every core
TB = 512                # token block (PSUM bank / moving free dim)
NTB = TOKS // TB        # 64
NBB = S // TB           # 16 blocks per batch sequence
KC = D // 128           # 8 contraction chunks of 128

USE_FP8 = True
WSCALE = 64.0           # w pre-scale so fp8 w values sit in the normal range
N_DUMMY = 16            # HAM pre-warm matmuls issued while startup DMAs fly

_CACHE = {}


def _build_nc(use_fp8):
    import concourse.bacc as bacc
    import concourse.mybir as mybir
    import concourse.tile as tile

    dt = mybir.dt
    AF = mybir.ActivationFunctionType
    ALU = mybir.AluOpType

    in_dt = dt.float8e4 if use_fp8 else dt.bfloat16
    inv = 1.0 / WSCALE if use_fp8 else 1.0

    nc = bacc.Bacc("TRN2", target_bir_lowering=False)
    # xt[tb, p, kc, s] = x_flat[tb*TB + s, kc*128 + p] — per-partition rows
    # are KC*TB contiguous bytes so each block DMA is dense 4 KB descriptors.
    xt = nc.dram_tensor("xt", (NTB, 128, KC, TB), in_dt, kind="ExternalInput")
    # wt[p, kc, g*128+c] = w_in[g*H + core*128 + c, kc*128 + p]
    wt = nc.dram_tensor("wt", (128, KC, 384), in_dt, kind="ExternalInput")
    # sums[p, b] = sum over batch b's tokens of the per-channel term
    out_sums = nc.dram_tensor("sums", (128, B), dt.float32, kind="ExternalOutput")

    with tile.TileContext(nc) as tc, ExitStack() as ctx:
        wpool = ctx.enter_context(tc.tile_pool(name="w", bufs=1))
        xpool = ctx.enter_context(tc.tile_pool(name="x", bufs=3))
        gpool = ctx.enter_context(tc.tile_pool(name="g", bufs=2))
        spool = ctx.enter_context(tc.tile_pool(name="s", bufs=1))
        psh = ctx.enter_context(tc.tile_pool(name="psh", bufs=2, space="PSUM"))
        psf = ctx.enter_context(tc.tile_pool(name="psf", bufs=2, space="PSUM"))
        psi = ctx.enter_context(tc.tile_pool(name="psi", bufs=2, space="PSUM"))

        slab = spool.tile([128, B, NBB], dt.float32)

        # HAM pre-warm: PE clock gate defaults to 1.2 GHz and opens after
        # ~3.4us of sustained activity. Start the activity window during the
        # startup DMAs; the first real matmuls continue it (running cold is
        # still faster than waiting for warm).
        dum = gpool.tile([128, 64], dt.bfloat16, tag="dum")
        nc.vector.memset(dum[:], 0.0)
        psd = psh.tile([128, TB], dt.float32, tag="ps", bufs=2)
        for _ in range(N_DUMMY):
            nc.tensor.matmul(psd[0:64, 0:64], dum[:], dum[:], start=True, stop=True)

        # Startup-critical DMAs in kb-pair pieces across both HWDGE queues:
        # the first matmul only waits for the first ~224 KB.
        w_all = wpool.tile([128, KC, 384], in_dt)
        for kb in range(KC // 2):
            nc.scalar.dma_start(
                w_all[:, 2 * kb : 2 * kb + 2, :], wt[:, 2 * kb : 2 * kb + 2, :]
            )
        x0 = xpool.tile([128, KC, TB], in_dt, tag="x")
        for kb in range(KC // 2):
            nc.sync.dma_start(
                x0[:, 2 * kb : 2 * kb + 2, :], xt[0, :, 2 * kb : 2 * kb + 2, :]
            )

        for tb in range(NTB):
            bb, ib = divmod(tb, NBB)
            if tb == 0:
                x_sb = x0
            else:
                x_sb = xpool.tile([128, KC, TB], in_dt, tag="x")
                nc.sync.dma_start(x_sb[:], xt[tb])

            ph = psh.tile([128, TB], dt.float32, tag="ps", bufs=2)
            pf = psf.tile([128, TB], dt.float32, tag="ps", bufs=2)
            pi = psi.tile([128, TB], dt.float32, tag="ps", bufs=2)

            def mm_group(out_ap, g):
                for kb in range(KC // 2):
                    nc.tensor.matmul(
                        out_ap,
                        w_all[:, 2 * kb : 2 * kb + 2, g * 128 : (g + 1) * 128],
                        x_sb[:, 2 * kb : 2 * kb + 2, :],
                        start=(kb == 0),
                        stop=(kb == KC // 2 - 1),
                        perf_mode=mybir.MatmulPerfMode.DoubleRow,
                    )

            # h first (feeds the 2-op ACT chain), i last (the only gate on
            # the post-last-matmul critical path: ti -> t -> readacc).
            mm_group(ph[:], 2)  # h gate
            mm_group(pf[:], 0)  # f gate
            mm_group(pi[:], 1)  # i gate

            th = gpool.tile([128, TB], dt.bfloat16, tag="th")
            h2 = gpool.tile([128, TB], dt.bfloat16, tag="h2")
            a = gpool.tile([128, TB], dt.bfloat16, tag="a")
            ti = gpool.tile([128, TB], dt.bfloat16, tag="ti")
            nc.scalar.activation(th[:], ph[:], AF.Tanh, scale=0.5 * inv)
            nc.scalar.activation(h2[:], ph[:], AF.Copy, scale=2.0 * inv, bias=1.0)
            nc.scalar.activation(a[:], pf[:], AF.Exp, scale=-inv)
            nc.scalar.activation(ti[:], pi[:], AF.Tanh, scale=0.5 * inv)

            # w2 = max(1+tanh(h/2), 1+2h);  r = (1+e^{-f}) * w2
            # t = (1+tanh(i/2)) * r, row-summed into slab[:, batch, block]
            w2 = gpool.tile([128, TB], dt.bfloat16, tag="w2")
            nc.vector.scalar_tensor_tensor(
                w2[:], th[:], 1.0, h2[:], op0=ALU.add, op1=ALU.max
            )
            r = gpool.tile([128, TB], dt.bfloat16, tag="r")
            nc.vector.scalar_tensor_tensor(
                r[:], a[:], 1.0, w2[:], op0=ALU.add, op1=ALU.mult
            )
            t = gpool.tile([128, TB], dt.bfloat16, tag="t")
            nc.vector.scalar_tensor_tensor(
                t[:],
                ti[:],
                1.0,
                r[:],
                op0=ALU.add,
                op1=ALU.mult,
                accum_out=slab[:, bb, ib : ib + 1],
            )

        red = spool.tile([128, B], dt.float32)
        nc.vector.tensor_reduce(red[:], slab[:], axis=mybir.AxisListType.X, op=ALU.add)
        nc.sync.dma_start(out_sums[:], red[:])

    nc.compile()
    return nc


def _get_nc():
    key = "fp8" if USE_FP8 else "bf16"
    if key not in _CACHE:
        _CACHE[key] = _build_nc(USE_FP8)
    return _CACHE[key]


def _softplus(v):
    return np.log1p(np.exp(-np.abs(v))) + np.maximum(v, 0.0)


def kernel(x, w_in, w_out, b_out, _return_results=False, _trace=False):
    from concourse.bass_utils import run_bass_kernel_spmd

    x = np.asarray(x)
    w_in = np.asarray(w_in)
    w_out = np.asarray(w_out)
    b_out = np.asarray(b_out)

    if USE_FP8:
        cast_dt = ml_dtypes.float8_e4m3  # TRN FP8_EXP4: max ±240, inf above

        def cast(a):
            return np.clip(a, -240.0, 240.0).astype(cast_dt)

        w_scaled = w_in * WSCALE
    else:
        cast_dt = ml_dtypes.bfloat16

        def cast(a):
            return a.astype(cast_dt)

        w_scaled = w_in

    # per-core weight pack: wt[p, kc, g*128+c] = w_scaled[g*H + core*128+c, kc*128+p]
    w5 = w_scaled.reshape(3, N_CORES, CH, KC, 128)
    wts = []
    for c in range(N_CORES):
        wc = np.ascontiguousarray(w5[:, c].transpose(3, 2, 0, 1))  # [128p, KC, 3, CH]
        wts.append(np.asarray(cast(wc)).reshape(128, KC, 384))

    # shared token pack: xt[tb, p, kc, s] = x_flat[tb*TB + s, kc*128 + p]
    xq = cast(x.reshape(TOKS, D))
    xt = np.ascontiguousarray(
        np.asarray(xq).reshape(NTB, TB, KC, 128).transpose(0, 3, 2, 1)
    )

    in_maps = [{"xt": xt, "wt": wts[c]} for c in range(N_CORES)]

    nc = _get_nc()
    # the first execution of a freshly compiled NEFF occasionally hits a
    # transient NRT exec error on this setup — retry once
    try:
        res = run_bass_kernel_spmd(
            nc, in_maps, core_ids=list(range(N_CORES)), trace=_trace
        )
    except Exception:
        import time as _time

        _time.sleep(2.0)
        res = run_bass_kernel_spmd(
            nc, in_maps, core_ids=list(range(N_CORES)), trace=False
        )

    # sums[p, b] per core -> channel h = core*128 + p
    Ssum = (
        np.concatenate([np.asarray(r["sums"]).T for r in res.results], axis=1).astype(
            np.float64
        )
        * 0.25
    )  # [B, H]

    # exact last-token factor in fp64 (host): log_f[S-1] = -softplus(diff[S-1])
    z_last = x[:, -1, :].astype(np.float64) @ w_in.astype(np.float64).T
    f_l, i_l = z_last[:, :H], z_last[:, H : 2 * H]
    diff_l = _softplus(-f_l) - _softplus(-i_l)
    h_last = np.exp(-_softplus(diff_l) + np.log(0.5 + Ssum))
    out = (h_last @ w_out.astype(np.float64).T + b_out.astype(np.float64)).astype(
        np.float32
    )
    if _return_results:
        return out, res
    return out
